# revision 31
# baseline (speedup 1.0000x reference)
"""Trainium2 Bass kernel for nn_ChoreographModel (conv stack + 2-layer LSTM + FC).

Strategy: pure data-parallel over 8 NeuronCores (batch 4096 -> 512/core).
Per core:
  conv1 with dw0/dw1 folded into one K=90 contraction (x replicated with a
  1-column shift host-side) + a K=45 pass for dw2 -> 2 matmuls per chunk,
  x streamed in pipelined subchunks on the two HWDGE queues (sync+scalar)
  while all weights load on the gpsimd SWDGE queue;
  maxpool+relu fused on DVE/ACT; conv2 (3x3x10->20) Toeplitz over
  (h1,c1)=90 partitions in two half-batches whose outputs are re-laid
  into per-timestep feature-major tiles by gpsimd/scalar/vector engine
  copies overlapped with conv2 compute; then a 7-step 2-layer LSTM
  computed entirely in transposed form zT=[gates, batch] and FC+relu.
  Matmul inputs bf16, accumulation fp32, cell state fp32.
"""

import sys
from contextlib import ExitStack

if "/opt/trn_rl_repo" not in sys.path:
    sys.path.insert(0, "/opt/trn_rl_repo")

import numpy as np
import ml_dtypes

BF16 = ml_dtypes.bfloat16
MM_NAME = "bfloat16"
MM_NP = BF16
N_CORES = 8

H = 512
NCLS = 256

# conv geometry (hardcoded from the model)
IH, IW, CI = 15, 80, 3
OH1, OW1, CO1 = 9, 78, 10   # after conv1
PW1 = 26                     # after pool1 (78/3)
OH2, OW2, CO2 = 7, 24, 20    # after conv2
PW2 = 8                      # after pool2 (24/3)
T = OH2                      # timesteps
F = PW2 * CO2                # 160 LSTM input features
K1 = IH * CI                 # 45  conv1 contraction rows (per dw)
M1 = OH1 * CO1               # 90  conv1 output rows
K2 = OH1 * CO1               # 90  conv2 contraction rows
M2A, M2B = 4 * CO2, 3 * CO2  # 80/60 conv2 output row groups (oh2 0-3 / 4-6)

NB1 = 6    # conv1 batch chunk (6*78=468 <= 512 psum bank)
NB2 = 21   # conv2 batch chunk (21*24=504 <= 512)
SUBB = 64  # conv1 x-streaming subchunk (batches)


def build_nc(B, nsteps=T, reps=1, hw_loop=0,
             ps1bufs=6, ps2bufs=3, zbufs=8, gbufs=3, xbufs=3):
    import concourse.bacc as bacc
    import concourse.tile as tile
    from concourse import mybir

    dt = mybir.dt
    AF = mybir.ActivationFunctionType
    MM = getattr(dt, MM_NAME)

    nc = bacc.Bacc("TRN2", target_bir_lowering=False, debug=False,
                   num_devices=N_CORES)

    x2_d = nc.dram_tensor("x2", [2 * K1, B * IW], MM, kind="ExternalInput")
    w1ta_d = nc.dram_tensor("w1ta", [2 * K1, M1], MM, kind="ExternalInput")
    w1tb_d = nc.dram_tensor("w1tb", [K1, M1], MM, kind="ExternalInput")
    cb1_d = nc.dram_tensor("cb1", [M1, 1], dt.float32, kind="ExternalInput")
    w2t_d = nc.dram_tensor("w2t", [K2, 3 * (M2A + M2B)], MM, kind="ExternalInput")
    cb2a_d = nc.dram_tensor("cb2a", [M2A, 1], dt.float32, kind="ExternalInput")
    cb2b_d = nc.dram_tensor("cb2b", [M2B, 1], dt.float32, kind="ExternalInput")
    w1l_d = nc.dram_tensor("w1l", [F, 4 * H], MM, kind="ExternalInput")
    u1l_d = nc.dram_tensor("u1l", [H, 4 * H], MM, kind="ExternalInput")
    w2l_d = nc.dram_tensor("w2l", [H, 4 * H], MM, kind="ExternalInput")
    u2l_d = nc.dram_tensor("u2l", [H, 4 * H], MM, kind="ExternalInput")
    bl_d = nc.dram_tensor("bl", [128, 32], dt.float32, kind="ExternalInput")
    fcw_d = nc.dram_tensor("fcw", [H, NCLS], MM, kind="ExternalInput")
    fcb_d = nc.dram_tensor("fcb", [128, 2], dt.float32, kind="ExternalInput")
    out_d = nc.dram_tensor("out", [NCLS, B], dt.float32, kind="ExternalOutput")
    # DRAM scratch for the conv2->LSTM feature shuffle: logical [F, T*B] in
    # the exact (w2,co) x (t,b) layout the LSTM input tiles need. DMA through
    # DRAM is the only engine-free way to fold the free w2 axis into
    # partitions (SBUF APs cannot cross partitions in a free dim, and
    # compute engines need 32-aligned partition bases).
    tsh_d = nc.dram_tensor("tshuf", [F, T * B], MM, kind="Internal")

    HB = B // 2          # conv2 half-batch
    nch2 = (HB + NB2 - 1) // NB2
    nsub = (B + SUBB - 1) // SUBB

    with tile.TileContext(nc) as tc:
        with tc.tile_pool(name="consts", bufs=1) as cp, \
             tc.tile_pool(name="seq", bufs=1) as seqp:

            # ---- persistent constants: all on the gpsimd SWDGE queue so the
            # HWDGE queues (sync+scalar) are free for the x stream. Conv
            # weights first (needed at t~2us), LSTM weights after.
            w1ta_s = cp.tile([2 * K1, M1], MM, tag="w1ta")
            nc.gpsimd.dma_start(w1ta_s[:], w1ta_d[:])
            w1tb_s = cp.tile([K1, M1], MM, tag="w1tb")
            nc.gpsimd.dma_start(w1tb_s[:], w1tb_d[:])
            cb1_s = cp.tile([M1, 1], dt.float32, tag="cb1")
            nc.gpsimd.dma_start(cb1_s[:], cb1_d[:])
            w2t_s = cp.tile([K2, 3 * (M2A + M2B)], MM, tag="w2t")
            nc.gpsimd.dma_start(w2t_s[:], w2t_d[:])
            cb2a_s = cp.tile([M2A, 1], dt.float32, tag="cb2a")
            nc.gpsimd.dma_start(cb2a_s[:], cb2a_d[:])
            cb2b_s = cp.tile([M2B, 1], dt.float32, tag="cb2b")
            nc.gpsimd.dma_start(cb2b_s[:], cb2b_d[:])
            bl_s = cp.tile([128, 32], dt.float32, tag="bl")
            nc.gpsimd.dma_start(bl_s[:], bl_d[:])
            fcb_s = cp.tile([128, 2], dt.float32, tag="fcb")
            nc.gpsimd.dma_start(fcb_s[:], fcb_d[:])

            # all big LSTM weights are DMA'd inside the body, after the x
            # stream, so neither the gpsimd queue (x second halves) nor the
            # scalar queue (conv ACT chain) is blocked early.
            w1a_s = cp.tile([128, 4 * H], MM, tag="w1a")
            w1b_s = cp.tile([F - 128, 4 * H], MM, tag="w1b")
            u1_s, w2_s, u2_s, fcw_s = [], [], [], []
            for k in range(4):
                u1_s.append(cp.tile([128, 4 * H], MM, tag=f"u1_{k}",
                                    name=f"u1_{k}"))
                w2_s.append(cp.tile([128, 4 * H], MM, tag=f"w2_{k}",
                                    name=f"w2_{k}"))
                u2_s.append(cp.tile([128, 4 * H], MM, tag=f"u2_{k}",
                                    name=f"u2_{k}"))
                fcw_s.append(cp.tile([128, NCLS], MM, tag=f"fcw_{k}",
                                     name=f"fcw_{k}"))

            import contextlib
            loop_cm = tc.For_i(0, hw_loop, 1) if hw_loop else contextlib.nullcontext()
            with loop_cm:
             for rep in range(reps):
                # ---- per-timestep LSTM input tiles (filled by conv2 copies)
                xhi_t = [seqp.tile([128, B], MM, tag=f"xhi{t}", name=f"xhi{t}")
                         for t in range(T)]
                xlo_t = [seqp.tile([F - 128, B], MM, tag=f"xlo{t}", name=f"xlo{t}")
                         for t in range(T)]

                # ---- conv1 + pool1 --------------------------------------------
                with tc.tile_pool(name="t1p", bufs=1) as t1p, \
                     tc.tile_pool(name="ctmp", bufs=3) as ctp:
                    t1_s = t1p.tile([M1, B * PW1], MM, tag="t1")
                    with tc.tile_pool(name="xin", bufs=xbufs) as xp, \
                         tc.tile_pool(name="ps1", bufs=ps1bufs, space="PSUM") as pp1:
                        xt = {}

                        def load_x(s):
                            sb0 = s * SUBB
                            nbs = min(SUBB, B - sb0)
                            th = xp.tile([2 * K1, SUBB * IW], MM, tag="xq",
                                         name=f"xq{s}")
                            half = (nbs // 2) * IW
                            # sync + gpsimd: the scalar queue must stay clear
                            # for the conv ACT chain (psum drain).
                            nc.sync.dma_start(
                                th[:, 0:half],
                                x2_d[:, sb0 * IW:sb0 * IW + half])
                            nc.gpsimd.dma_start(
                                th[:, half:nbs * IW],
                                x2_d[:, sb0 * IW + half:(sb0 + nbs) * IW])
                            xt[s] = th

                        for s in range(min(2, nsub)):
                            load_x(s)
                        for s in range(nsub):
                            if s + 2 < nsub:
                                load_x(s + 2)
                            sb0 = s * SUBB
                            nbs = min(SUBB, B - sb0)
                            x2_r = xt[s][:].rearrange("p (b w) -> p b w", w=IW)
                            for bl0 in range(0, nbs, NB1):
                                nb = min(NB1, nbs - bl0)
                                b0 = sb0 + bl0
                                ps = pp1.tile([M1, nb * OW1], dt.float32,
                                              tag="ps1", name="ps1")
                                # dw0+dw1 folded (x replicated with 1-col
                                # shift in rows 45..89), then dw2 on the
                                # unshifted rows at column offset 2.
                                nc.tensor.matmul(
                                    ps[:], w1ta_s[:],
                                    x2_r[:, bl0:bl0 + nb, 0:OW1],
                                    start=True, stop=False)
                                nc.tensor.matmul(
                                    ps[:], w1tb_s[:],
                                    x2_r[0:K1, bl0:bl0 + nb, 2:2 + OW1],
                                    start=False, stop=True)
                                # relu+bias commute with max-pool: ACT drains
                                # PSUM to bf16 first (freeing the bank fast),
                                # then the DVE pools at 2x bf16 rate straight
                                # into t1.
                                stg = ctp.tile([M1, nb * OW1], MM,
                                               tag="stg1", name="stg1")
                                nc.scalar.activation(
                                    stg[:], ps[:], AF.Relu, bias=cb1_s[:])
                                nc.vector.reduce_max(
                                    t1_s[:, b0 * PW1:(b0 + nb) * PW1],
                                    stg[:].rearrange("p (b q k) -> p b q k",
                                                     q=PW1, k=3),
                                    axis=mybir.AxisListType.X,
                                )
                            del xt[s]

                        # big LSTM weights behind the x stream (needed only by
                        # t~95us): sync + gpsimd, never the scalar queue.
                        nc.sync.dma_start(w1a_s[:], w1l_d[0:128, :])
                        nc.gpsimd.dma_start(w1b_s[:], w1l_d[128:F, :])
                        for k in range(4):
                            nc.sync.dma_start(u1_s[k][:],
                                              u1l_d[128 * k:128 * (k + 1), :])
                            nc.gpsimd.dma_start(w2_s[k][:],
                                                w2l_d[128 * k:128 * (k + 1), :])
                            nc.sync.dma_start(u2_s[k][:],
                                              u2l_d[128 * k:128 * (k + 1), :])
                            nc.gpsimd.dma_start(fcw_s[k][:],
                                                fcw_d[128 * k:128 * (k + 1), :])

                    # ---- conv2 + pool2, two half-batches; shuffle into the
                    # per-timestep tiles via engine copies overlapped with
                    # the next half's compute ------------------------------
                    t1_r = t1_s[:].rearrange("p (b w) -> p b w", w=PW1)

                    # tshuf viewed [t, co, w2, b]; row of the logical [F, T*B]
                    # layout is w2*CO2+co, column t*B+b.
                    td4 = tsh_d[:].rearrange("(w c) (t b) -> t c w b",
                                             c=CO2, b=B)

                    def shuffle_half(hf, t2a_s, t2b_s):
                        # one DMA per (t2 tile, timestep) scatters the 20-row
                        # t-slice into the shuffled DRAM layout (DMA APs are
                        # limited to 3 dims, so the t axis is peeled). For the
                        # second half, the t0/t1 readbacks are interleaved
                        # right behind their writes on the HWDGE queues so the
                        # LSTM can start ~2us after the last conv2 act; later
                        # timesteps read back on the idle gpsimd queue.
                        for t in range(T):
                            src = t2a_s if t < 4 else t2b_s
                            rs = (t % 4) * CO2 if t < 4 else (t - 4) * CO2
                            if hf == 1:
                                # only the LSTM-gating t0/t1 traffic stays on
                                # the fast HWDGE queues; the rest rides gpsimd
                                q = (nc.sync if t == 0 else
                                     nc.scalar if t == 1 else nc.gpsimd)
                            else:
                                # scalar queue stays clear for conv2 ACTs
                                q = nc.sync if t % 2 == 0 else nc.gpsimd
                            q.dma_start(
                                td4[t:t + 1, :, :, hf * HB:(hf + 1) * HB],
                                src[rs:rs + CO2].rearrange(
                                    "p (w b) -> p w b", b=HB))
                            if hf == 0 and t == 0:
                                # t0's first-half columns read back early so
                                # the LSTM's half-0 xW matmuls can fill the
                                # PE gap while the h1 write/read completes.
                                nc.sync.dma_start(
                                    xhi_t[0][:, 0:HB], tsh_d[0:128, 0:HB])
                                nc.sync.dma_start(
                                    xlo_t[0][:, 0:HB], tsh_d[128:F, 0:HB])
                            if hf == 1 and t <= 1:
                                c0, c1 = (t * B + HB, (t + 1) * B) if t == 0 \
                                    else (t * B, (t + 1) * B)
                                q.dma_start(xhi_t[t][:, c0 - t * B:c1 - t * B],
                                            tsh_d[0:128, c0:c1])
                                q.dma_start(xlo_t[t][:, c0 - t * B:c1 - t * B],
                                            tsh_d[128:F, c0:c1])

                    # t2 halves live in the long-lived seq pool: the shuffle
                    # copies read them asynchronously deep into the LSTM
                    # phase, and a closing pool here would gate the LSTM's
                    # state/gates pool allocation on those copies.
                    with tc.tile_pool(name="ps2", bufs=ps2bufs, space="PSUM") as pp2:
                        for hf in range(2):
                            t2a_s = seqp.tile([M2A, PW2 * HB], MM, tag=f"t2a{hf}",
                                              name=f"t2a{hf}")
                            t2b_s = seqp.tile([M2B, PW2 * HB], MM, tag=f"t2b{hf}",
                                              name=f"t2b{hf}")
                            t2a_r = t2a_s[:].rearrange("p (w b) -> p b w", b=HB)
                            t2b_r = t2b_s[:].rearrange("p (w b) -> p b w", b=HB)
                            for c in range(nch2):
                                b0 = c * NB2
                                nb = min(NB2, HB - b0)
                                gb0 = hf * HB + b0
                                for grp, (off, M, cb_s, t2_r) in enumerate([
                                    (0, M2A, cb2a_s, t2a_r),
                                    (M2A, M2B, cb2b_s, t2b_r),
                                ]):
                                    ps = pp2.tile([M, nb * OW2], dt.float32,
                                                  tag=f"ps2_{grp}", name="ps2")
                                    for dw in range(3):
                                        nc.tensor.matmul(
                                            ps[:],
                                            w2t_s[:, dw * 140 + off:
                                                  dw * 140 + off + M],
                                            t1_r[:, gb0:gb0 + nb, dw:dw + OW2],
                                            start=(dw == 0), stop=(dw == 2),
                                        )
                                    stg = ctp.tile([M, nb * OW2], MM,
                                                   tag=f"stg2_{grp}",
                                                   name="stg2")
                                    nc.scalar.activation(
                                        stg[:], ps[:], AF.Relu, bias=cb_s[:])
                                    nc.vector.reduce_max(
                                        t2_r[:, b0:b0 + nb, :],
                                        stg[:].rearrange("p (b q k) -> p b q k",
                                                         q=PW2, k=3),
                                        axis=mybir.AxisListType.X,
                                    )
                            shuffle_half(hf, t2a_s, t2b_s)

                        # read back the remaining per-timestep LSTM input
                        # tiles on the idle gpsimd queue (t0/t1 were read
                        # inline behind their writes above).
                        for t in range(2, T):
                            nc.gpsimd.dma_start(xhi_t[t][:],
                                                tsh_d[0:128, t * B:(t + 1) * B])
                            nc.gpsimd.dma_start(xlo_t[t][:],
                                                tsh_d[128:F, t * B:(t + 1) * B])

                # ---- LSTM ----------------------------------------------------
                # h tiles are ping-pong buffered across steps: within step t the
                # U-matmuls of every j-tile read the step-(t-1) h, while the cell
                # update writes the step-t h into the other buffer.
                ls = ExitStack()
                stp = ls.enter_context(tc.tile_pool(name="state", bufs=1))
                gp = ls.enter_context(tc.tile_pool(name="gates", bufs=gbufs))
                h1_t = [[stp.tile([128, B], MM, tag=f"h1_{p}_{j}", name=f"h1_{p}_{j}")
                         for j in range(4)] for p in range(2)]
                h2_t = [[stp.tile([128, B], MM, tag=f"h2_{p}_{j}", name=f"h2_{p}_{j}")
                         for j in range(4)] for p in range(2)]
                c1_t = [stp.tile([128, B], dt.float32, tag=f"c1_{j}", name=f"c1_{j}")
                        for j in range(4)]
                c2_t = [stp.tile([128, B], dt.float32, tag=f"c2_{j}", name=f"c2_{j}")
                        for j in range(4)]

                with tc.tile_pool(name="zps", bufs=zbufs, space="PSUM") as zpp:
                    for t in range(nsteps):
                        h1_prev, h1_new = h1_t[t % 2], h1_t[(t + 1) % 2]
                        h2_prev, h2_new = h2_t[t % 2], h2_t[(t + 1) % 2]
                        for layer in range(2):
                            h_prev = h1_prev if layer == 0 else h2_prev
                            h_new = h1_new if layer == 0 else h2_new
                            c_t = c1_t if layer == 0 else c2_t
                            u_s = u1_s if layer == 0 else u2_s
                            bcol = 0 if layer == 0 else 16
                            s_g4 = {}
                            for j in range(4):
                                for gi, gname in enumerate("ifgo"):
                                    m = gi * 4 + j
                                    col = gi * H + j * 128
                                    ps = zpp.tile([128, B], dt.float32, tag="z")
                                    if layer == 0:
                                        nc.tensor.matmul(
                                            ps[:], w1a_s[:, col:col + 128],
                                            xhi_t[t][:],
                                            start=True, stop=False)
                                        nc.tensor.matmul(
                                            ps[:], w1b_s[:, col:col + 128],
                                            xlo_t[t][:],
                                            start=False, stop=(t == 0))
                                        if t > 0:
                                            for k in range(4):
                                                nc.tensor.matmul(
                                                    ps[:], u_s[k][:, col:col + 128],
                                                    h_prev[k][:],
                                                    start=False, stop=(k == 3))
                                    else:
                                        if t > 0:
                                            for k in range(4):
                                                nc.tensor.matmul(
                                                    ps[:], u_s[k][:, col:col + 128],
                                                    h_prev[k][:],
                                                    start=(k == 0), stop=False)
                                        for k in range(4):
                                            nc.tensor.matmul(
                                                ps[:], w2_s[k][:, col:col + 128],
                                                h1_new[k][:],
                                                start=(t == 0 and k == 0),
                                                stop=(k == 3))
                                    func = AF.Tanh if gname == "g" else AF.Sigmoid
                                    s = gp.tile([128, B], dt.float32, tag=f"s_{gname}")
                                    nc.scalar.activation(
                                        s[:], ps[:], func,
                                        bias=bl_s[:, bcol + m:bcol + m + 1])
                                    s_g4[gname] = s
                                si, sf, sg, so = (s_g4[g] for g in "ifgo")
                                if t == 0:
                                    nc.vector.tensor_mul(c_t[j][:], si[:], sg[:])
                                else:
                                    tig = gp.tile([128, B], dt.float32, tag="tig")
                                    nc.vector.tensor_mul(tig[:], si[:], sg[:])
                                    nc.vector.tensor_mul(c_t[j][:], c_t[j][:], sf[:])
                                    nc.vector.tensor_add(c_t[j][:], c_t[j][:], tig[:])
                                tc_ = gp.tile([128, B], dt.float32, tag="tc")
                                nc.scalar.activation(tc_[:], c_t[j][:], AF.Tanh)
                                nc.vector.tensor_mul(h_new[j][:], so[:], tc_[:])

                    # ---- FC + relu ------------------------------------------
                    for mo in range(2):
                        ps = zpp.tile([128, B], dt.float32, tag="z")
                        for k in range(4):
                            nc.tensor.matmul(
                                ps[:], fcw_s[k][:, mo * 128:(mo + 1) * 128],
                                h2_t[nsteps % 2][k][:], start=(k == 0), stop=(k == 3))
                        o_s = gp.tile([128, B], dt.float32, tag="o")
                        nc.scalar.activation(o_s[:], ps[:], AF.Relu,
                                             bias=fcb_s[:, mo:mo + 1])
                        nc.sync.dma_start(out_d[mo * 128:(mo + 1) * 128, :], o_s[:])
                ls.close()

    nc.finalize()
    return nc


def prep_consts(conv1_w, conv1_b, conv2_w, conv2_b, W1, U1, b1, W2, U2, b2,
                fc_w, fc_b):
    w1ta = np.zeros((2 * K1, M1), np.float32)
    w1tb = np.zeros((K1, M1), np.float32)
    for oh in range(OH1):
        for dh in range(7):
            r = (oh + dh) * CI
            w1ta[r:r + CI, oh * CO1:(oh + 1) * CO1] = conv1_w[dh, 0]
            w1ta[K1 + r:K1 + r + CI, oh * CO1:(oh + 1) * CO1] = conv1_w[dh, 1]
            w1tb[r:r + CI, oh * CO1:(oh + 1) * CO1] = conv1_w[dh, 2]
    w2t = np.zeros((K2, 3 * (M2A + M2B)), np.float32)
    for dw in range(3):
        for oh in range(OH2):
            for dh in range(3):
                w2t[(oh + dh) * CO1:(oh + dh + 1) * CO1,
                    dw * 140 + oh * CO2:dw * 140 + (oh + 1) * CO2] = conv2_w[dh, dw]
    return {
        "w1ta": w1ta.astype(MM_NP),
        "w1tb": w1tb.astype(MM_NP),
        "cb1": np.tile(conv1_b, OH1)[:, None].astype(np.float32),
        "w2t": w2t.astype(MM_NP),
        "cb2a": np.tile(conv2_b, 4)[:, None].astype(np.float32),
        "cb2b": np.tile(conv2_b, 3)[:, None].astype(np.float32),
        "w1l": np.asarray(W1, np.float32).astype(MM_NP),
        "u1l": np.asarray(U1, np.float32).astype(MM_NP),
        "w2l": np.asarray(W2, np.float32).astype(MM_NP),
        "u2l": np.asarray(U2, np.float32).astype(MM_NP),
        "bl": np.concatenate([np.asarray(b1).reshape(16, 128).T,
                              np.asarray(b2).reshape(16, 128).T],
                             axis=1).astype(np.float32),
        "fcw": np.asarray(fc_w, np.float32).astype(MM_NP),
        "fcb": np.asarray(fc_b).reshape(2, 128).T.astype(np.float32),
    }


def prep_x(x_shard):
    B = x_shard.shape[0]
    top = np.asarray(x_shard, np.float32).transpose(1, 3, 0, 2).reshape(K1, B, IW)
    x2 = np.zeros((2 * K1, B, IW), np.float32)
    x2[:K1] = top
    x2[K1:, :, :-1] = top[:, :, 1:]
    return np.ascontiguousarray(x2.reshape(2 * K1, B * IW)).astype(MM_NP)


_NC_CACHE = {}


def _get_nc(B):
    if B not in _NC_CACHE:
        _NC_CACHE[B] = build_nc(B)
    return _NC_CACHE[B]


def kernel(**inputs):
    from concourse.bass_utils import run_bass_kernel_spmd

    x = np.asarray(inputs["x"])
    Bfull = x.shape[0]
    B = Bfull // N_CORES
    nc = _get_nc(B)
    consts = prep_consts(**{k: np.asarray(v) for k, v in inputs.items()
                            if k != "x"})
    in_maps = []
    for c in range(N_CORES):
        m = dict(consts)
        m["x2"] = prep_x(x[c * B:(c + 1) * B])
        in_maps.append(m)
    res = run_bass_kernel_spmd(nc, in_maps, list(range(N_CORES)))
    out = np.concatenate(
        [res.results[c]["out"].T for c in range(N_CORES)], axis=0)
    return np.ascontiguousarray(out.astype(np.float32))
